# revision 7
# baseline (speedup 1.0000x reference)
"""Multi-head causal self-attention (B=2, S=2048, D=1024, H=16) on 8 TRN2 cores.

Sharding: core = b*4 + hg  (b in {0,1} batch, hg in {0..3} head-group of 4 heads).
Per core: project qT/kT (pair-packed [128, S], bf16) and v ([S, 64] blocks, bf16),
compute transposed scores S^T = K Q^T per head (k on partitions), causal mask
added in PSUM via a -1e9-triangle accumulate-matmul (PE, replaces the DVE
tri-multiply), exp on ScalarE (bf16 out, one call per k-tile covering both
packed heads via a 3D AP), PV matmul with a ones-column appended to V so row 64
of the accumulator is the softmax sum, normalization = fp32 reciprocal straight
from PSUM + bf16 numerator cast + partition-broadcast multiply, then the
partial output projection streamed out per 128-token block. All tile pools are
opened once (no mid-kernel pool barriers); PSUM: 2x scores (2 banks) + 2 o_ps
(1 bank) + 2 shared projection banks. Host sums the 4 per-batch partials and
adds (b_v @ w_o.T + b_o); b_k is dropped (softmax invariant to per-query
constants); b_q applied on-device. Matmul operands bf16; accumulation fp32.
"""

import numpy as np
import ml_dtypes

import concourse.bass as bass
import concourse.mybir as mybir
import concourse.tile as tile
from concourse import bacc
from concourse.bass_utils import run_bass_kernel_spmd

B, S, D, H, DK = 2, 2048, 1024, 16, 64
N_CORES = 8
F32 = mybir.dt.float32
BF16 = mybir.dt.bfloat16
NPBF = ml_dtypes.bfloat16
AF = mybir.ActivationFunctionType
NEG_BIG = -1.0e9
V2_NORM = True     # norm path with bf16 numerator + mixed-dtype multiply
V2_MASK = True     # causal mask via PE accumulate-matmul
V2_EXP3D = True    # single 3D-AP exp call on diag tiles


def _build(debug=False):
    nc = bacc.Bacc("TRN2", target_bir_lowering=False, debug=False,
                   num_devices=N_CORES)
    xT = nc.dram_tensor("xT", [D, S], BF16, kind="ExternalInput").ap()
    wqT = nc.dram_tensor("wqT", [D, 256], BF16, kind="ExternalInput").ap()
    wkT = nc.dram_tensor("wkT", [D, 256], BF16, kind="ExternalInput").ap()
    wvT = nc.dram_tensor("wvT", [D, 256], BF16, kind="ExternalInput").ap()
    woT = nc.dram_tensor("woT", [256, D], BF16, kind="ExternalInput").ap()
    bq2 = nc.dram_tensor("bq2", [128, 2], F32, kind="ExternalInput").ap()
    mtri = nc.dram_tensor("mtri", [128, 128], BF16, kind="ExternalInput").ap()
    ident = nc.dram_tensor("ident", [128, 128], BF16, kind="ExternalInput").ap()
    y = nc.dram_tensor("y", [S, D], BF16, kind="ExternalOutput").ap()
    dbg = {}
    if debug:
        for nm, shp in [("qT", [128, 2, S]), ("kT", [128, 2, S]),
                        ("vv", [128, 16, 260]), ("oT", [128, 2, S])]:
            dbg[nm] = nc.dram_tensor(nm, shp, BF16, kind="ExternalOutput").ap()

    NQC = 4          # q-chunks of 512
    QC = 512
    NKT = S // 128   # k tiles

    with tile.TileContext(nc) as tc, \
            nc.allow_low_precision(reason="bf16 attention kernel"):
        with (
            tc.tile_pool(name="persist", bufs=1) as persist,
            tc.tile_pool(name="kqv", bufs=2) as kqv,
            tc.tile_pool(name="xw", bufs=1) as xw,
            tc.tile_pool(name="ep", bufs=4) as ep,
            tc.tile_pool(name="rp", bufs=6) as rp,
            tc.tile_pool(name="fsb", bufs=4) as fsb,
            tc.tile_pool(name="sq", bufs=2, space="PSUM") as sqp,
            tc.tile_pool(name="ops", bufs=1, space="PSUM") as opp,
            tc.tile_pool(name="xps", bufs=2, space="PSUM") as xps,
        ):
            qT_sb = [kqv.tile([128, S], BF16, tag="qT", name=f"qT{p}") for p in range(2)]
            kT_sb = [kqv.tile([128, S], BF16, tag="kT", name=f"kT{p}") for p in range(2)]
            v_sb = [persist.tile([128, 4 * 65], BF16, tag=f"v{t}", name=f"v{t}")
                    for t in range(NKT)]
            outT_sb = [persist.tile([128, S], BF16, tag=f"oT{p}", name=f"oTs{p}")
                       for p in range(2)]
            wo_sb = [persist.tile([128, D], BF16, tag=f"wo{p}", name=f"wo{p}")
                     for p in range(2)]
            mask_sb = persist.tile([128, 128], BF16, tag="mtri")
            ident_sb = persist.tile([128, 128], BF16, tag="ident")
            bq_sb = persist.tile([128, 2], F32, tag="bq")

            xt = [xw.tile([128, S], BF16, tag=f"x{c}", name=f"xt{c}") for c in range(8)]
            wq_sb = [xw.tile([128, 256], BF16, tag=f"wq{c}", name=f"wqs{c}") for c in range(8)]
            wk_sb = [xw.tile([128, 256], BF16, tag=f"wk{c}", name=f"wks{c}") for c in range(8)]
            wv_sb = [xw.tile([128, 256], BF16, tag=f"wv{c}", name=f"wvs{c}") for c in range(8)]
            for c in range(8):
                nc.sync.dma_start(out=xt[c], in_=xT[c * 128:(c + 1) * 128, :])
            for c in range(8):
                nc.scalar.dma_start(out=wq_sb[c], in_=wqT[c * 128:(c + 1) * 128, :])
                nc.scalar.dma_start(out=wk_sb[c], in_=wkT[c * 128:(c + 1) * 128, :])
            for c in range(8):
                nc.gpsimd.dma_start(out=wv_sb[c], in_=wvT[c * 128:(c + 1) * 128, :])
            nc.gpsimd.dma_start(out=bq_sb, in_=bq2)
            nc.gpsimd.dma_start(out=mask_sb, in_=mtri)
            nc.gpsimd.dma_start(out=ident_sb, in_=ident)
            for p in range(2):
                nc.gpsimd.dma_start(out=wo_sb[p], in_=woT[p * 128:(p + 1) * 128, :])

            def qk_chain(p, j, which):
                ps = xps.tile([128, QC], F32, tag="pj", name="ps")
                w_sb = wq_sb if which == "q" else wk_sb
                for c in range(8):
                    nc.tensor.matmul(
                        ps, w_sb[c][:, p * 128:(p + 1) * 128],
                        xt[c][:, j * QC:(j + 1) * QC],
                        start=(c == 0), stop=(c == 7))
                if which == "q":
                    nc.vector.tensor_scalar_add(
                        qT_sb[p][:, j * QC:(j + 1) * QC], ps, bq_sb[:, p:p + 1])
                else:
                    nc.vector.tensor_copy(kT_sb[p][:, j * QC:(j + 1) * QC], ps)

            def v_chain(t):
                ps_v = xps.tile([128, QC], F32, tag="pj", name="ps_v")
                for c in range(8):
                    nc.tensor.matmul(
                        ps_v[:, 0:256], xt[c][:, t * 128:(t + 1) * 128], wv_sb[c],
                        start=(c == 0), stop=(c == 7))
                v_view = v_sb[t].rearrange("p (h w) -> p h w", w=65)
                nc.vector.memset(v_view[:, :, 64:65], 1.0)
                nc.vector.tensor_copy(
                    v_view[:, :, 0:64],
                    ps_v[:, 0:256].rearrange("p (h w) -> p h w", w=64))

            norm_rest = []

            def emit_norm(p_, q0_, o_ps_):
                for s in range(2):
                    if V2_NORM:
                        t_cp = rp.tile([64, QC], BF16, tag="ocp", name="t_cp",
                                       bufs=4)
                        sums = rp.tile([1, QC], F32, tag="rc", name="sums",
                                       bufs=4)
                        nc.vector.tensor_copy(sums, o_ps_[s][64:65, :])
                        nc.vector.tensor_copy(t_cp, o_ps_[s][0:64, :])
                        norm_rest.append(
                            lambda p_=p_, q0_=q0_, s=s, t_cp=t_cp, sums=sums:
                            finish_norm(p_, q0_, s, t_cp, sums))
                    else:
                        o_cp = rp.tile([64, QC], F32, tag="ocp", name="o_cp",
                                       bufs=4)
                        sums = rp.tile([1, QC], F32, tag="rc", name="sums",
                                       bufs=4)
                        nc.vector.tensor_copy(sums, o_ps_[s][64:65, :])
                        nc.vector.tensor_copy(o_cp, o_ps_[s][0:64, :])
                        norm_rest.append(
                            lambda p_=p_, q0_=q0_, s=s, o_cp=o_cp, sums=sums:
                            finish_norm_v1(p_, q0_, s, o_cp, sums))

            def finish_norm(p_, q0_, s, t_cp, sums):
                recip = rp.tile([1, QC], F32, tag="recip2", name="recip")
                nc.vector.reciprocal_approx_fast(out=recip, in_=sums)
                bc = rp.tile([64, QC], F32, tag="bc", name="bc")
                nc.gpsimd.partition_broadcast(bc, recip)
                nc.vector.tensor_mul(
                    outT_sb[p_][s * 64:(s + 1) * 64, q0_:q0_ + QC],
                    t_cp, bc)

            def finish_norm_v1(p_, q0_, s, o_cp, sums):
                recip = rp.tile([1, QC], F32, tag="recip2", name="recip")
                nc.vector.reciprocal_approx_fast(out=recip, in_=sums)
                bc = rp.tile([64, QC], F32, tag="bc", name="bc")
                nc.gpsimd.partition_broadcast(bc, recip)
                nc.vector.tensor_mul(
                    outT_sb[p_][s * 64:(s + 1) * 64, q0_:q0_ + QC],
                    o_cp, bc)

            def emit_pair(p, fillers):
                for qc in range(NQC):
                    q0 = qc * QC
                    nkt = 4 * qc + 4
                    o_ps = [opp.tile([65, QC], F32, tag=f"o{s}", name=f"ops{s}")
                            for s in range(2)]
                    pend = None
                    for kt in range(nkt):
                        o = kt * 128 - q0
                        diag = o >= 0
                        lo = o if diag else 0
                        s_ab = sqp.tile([128, 2 * QC], F32, tag="sq", name="s_ab")
                        for s in range(2):
                            half = s * QC
                            nc.tensor.matmul(
                                s_ab[:, half + lo:half + QC],
                                kT_sb[p][s * 64:(s + 1) * 64,
                                         kt * 128:(kt + 1) * 128],
                                qT_sb[p][s * 64:(s + 1) * 64,
                                         q0 + lo:q0 + QC],
                                start=True, stop=(not diag),
                                tile_position=(s * 64, 0),
                                skip_group_check=True)
                        if diag:
                            for s in range(2):
                                nc.tensor.matmul(
                                    s_ab[:, s * QC + o:s * QC + o + 128],
                                    mask_sb, ident_sb,
                                    start=False, stop=True,
                                    skip_group_check=True)
                        e_ab = ep.tile([128, 2 * QC], BF16, tag="e", name="e_ab")
                        if diag:
                            e3 = e_ab.rearrange("p (s n) -> p s n", n=QC)
                            s3 = s_ab.rearrange("p (s n) -> p s n", n=QC)
                            nc.scalar.activation(
                                e3[:, :, lo:QC], s3[:, :, lo:QC],
                                AF.Exp, scale=0.125)
                        else:
                            nc.scalar.activation(e_ab, s_ab, AF.Exp, scale=0.125)
                        if kt == 0:
                            while norm_rest:
                                norm_rest.pop(0)()
                        if fillers is not None:
                            fillers(qc, kt, nkt)
                        if pend is not None:
                            _kt, _e, _lo = pend
                            for s in range(2):
                                hb = 2 * p + s
                                nc.tensor.matmul(
                                    o_ps[s][:, _lo:QC],
                                    v_sb[_kt][:, hb * 65:(hb + 1) * 65],
                                    _e[:, s * QC + _lo:(s + 1) * QC],
                                    start=(_kt == 0), stop=False,
                                    skip_group_check=True)
                        pend = (kt, e_ab, lo)
                    _kt, _e, _lo = pend
                    for s in range(2):
                        hb = 2 * p + s
                        nc.tensor.matmul(
                            o_ps[s][:, _lo:QC],
                            v_sb[_kt][:, hb * 65:(hb + 1) * 65],
                            _e[:, s * QC + _lo:(s + 1) * QC],
                            start=False, stop=True,
                            skip_group_check=True)
                    emit_norm(p, q0, o_ps)
                    yield qc
                while norm_rest:
                    norm_rest.pop(0)()

            # ---- head: j0 + v0-3 for pair 0 gate the first attention chunk ----
            qk_chain(0, 0, "q")
            qk_chain(0, 0, "k")
            for t in range(4):
                v_chain(t)

            # ---- pair 0; remaining projections as fillers ----
            fillers = []
            for j in range(1, 4):
                fillers.append(lambda j=j: qk_chain(0, j, "q"))
                fillers.append(lambda j=j: qk_chain(0, j, "k"))
                for t in range(4 * j, 4 * j + 4):
                    fillers.append(lambda t=t: v_chain(t))
            # qc0: j1+v4-7, qc1: j2+v8-11, qc2: j3+v12-15,
            # qc3: pair-1 j0+j1 projection chains
            sched = {0: fillers[0:6], 1: fillers[6:12], 2: fillers[12:18]}
            sched[3] = []
            for j in range(2):
                sched[3].append(lambda j=j: qk_chain(1, j, "q"))
                sched[3].append(lambda j=j: qk_chain(1, j, "k"))
            queues = [list(sched[qc]) for qc in range(4)]

            def filler_pop(qc, kt, nkt):
                q = queues[qc]
                rem_slots = nkt - kt
                while q and len(q) >= rem_slots:
                    q.pop(0)()
                if q:
                    q.pop(0)()

            for _qc in emit_pair(0, filler_pop):
                while queues[_qc]:
                    queues[_qc].pop(0)()

            if debug:
                for p in range(2):
                    nc.sync.dma_start(out=dbg["qT"][:, p, :], in_=qT_sb[p])
                    nc.sync.dma_start(out=dbg["kT"][:, p, :], in_=kT_sb[p])
                for t in range(NKT):
                    nc.sync.dma_start(out=dbg["vv"][:, t, :], in_=v_sb[t])

            # ---- pair 1: pair-1 j2/j3 chains + output projection as fillers ----
            def c_unit(qt, oc):
                f_ps = xps.tile([128, QC], F32, tag="pj", name="f_ps")
                for p in range(2):
                    nc.tensor.matmul(
                        f_ps, outT_sb[p][:, qt * 128:(qt + 1) * 128],
                        wo_sb[p][:, oc * 512:(oc + 1) * 512],
                        start=(p == 0), stop=(p == 1))
                f_sb = fsb.tile([128, QC], BF16, tag="f", name="f_sb")
                nc.vector.tensor_copy(f_sb, f_ps)
                nc.sync.dma_start(
                    out=y[qt * 128:(qt + 1) * 128,
                          oc * 512:(oc + 1) * 512],
                    in_=f_sb)

            cqueues = [[] for _ in range(4)]
            for j in (2, 3):
                cqueues[j - 2].append(lambda j=j: qk_chain(1, j, "q"))
                cqueues[j - 2].append(lambda j=j: qk_chain(1, j, "k"))

            def c_pop(qc, kt, nkt):
                q = cqueues[qc]
                rem_slots = nkt - kt
                while q and len(q) >= rem_slots:
                    q.pop(0)()
                if q:
                    q.pop(0)()

            for qc in emit_pair(1, c_pop):
                units = []
                for qt in range(qc * 4, (qc + 1) * 4):
                    for oc in range(2):
                        units.append(
                            lambda qt=qt, oc=oc: c_unit(qt, oc))
                if qc < 3:
                    cqueues[qc + 1].extend(units)
                else:
                    while norm_rest:
                        norm_rest.pop(0)()
                    for u in units:
                        u()
            for q in cqueues:
                while q:
                    q.pop(0)()

            if debug:
                for p in range(2):
                    nc.sync.dma_start(out=dbg["oT"][:, p, :], in_=outT_sb[p])

    nc.compile()
    return nc


_cached = {}


def _get_nc(debug=False):
    key = bool(debug)
    if key not in _cached:
        _cached[key] = _build(debug)
    return _cached[key]


def _prep_inputs(x, w_q, b_q, w_k, w_v):
    mtri = (np.triu(np.ones((128, 128), np.float32), 1) * NEG_BIG).astype(NPBF)
    ident = np.eye(128, dtype=np.float32).astype(NPBF)
    wqT_f = np.ascontiguousarray(w_q.T).astype(NPBF)
    wkT_f = np.ascontiguousarray(w_k.T).astype(NPBF)
    wvT_f = np.ascontiguousarray(w_v.T).astype(NPBF)
    in_maps = []
    for core in range(N_CORES):
        b, hg = divmod(core, 4)
        cs = slice(hg * 256, (hg + 1) * 256)
        in_maps.append({
            "xT": np.ascontiguousarray(x[b].T).astype(NPBF),
            "wqT": np.ascontiguousarray(wqT_f[:, cs]),
            "wkT": np.ascontiguousarray(wkT_f[:, cs]),
            "wvT": np.ascontiguousarray(wvT_f[:, cs]),
            "bq2": np.ascontiguousarray(
                b_q[hg * 256:(hg + 1) * 256].reshape(2, 128).T.astype(np.float32)),
            "mtri": mtri,
            "ident": ident,
        })
    return in_maps


def _numpy_reference(x, attention_mask, w_q, b_q, w_k, b_k, w_v, b_v, w_o, b_o):
    x = x.astype(np.float64)
    q = (x @ w_q.T + b_q).reshape(B, S, H, DK).transpose(0, 2, 1, 3)
    k = (x @ w_k.T + b_k).reshape(B, S, H, DK).transpose(0, 2, 1, 3)
    v = (x @ w_v.T + b_v).reshape(B, S, H, DK).transpose(0, 2, 1, 3)
    scores = np.einsum("bhqd,bhkd->bhqk", q, k) / np.sqrt(DK)
    causal = np.tril(np.ones((S, S), bool))
    mask = causal[None, None] & (attention_mask[:, None, None, :] != 0)
    scores = np.where(mask, scores, -np.inf)
    scores -= scores.max(-1, keepdims=True)
    e = np.exp(scores)
    attn = e / e.sum(-1, keepdims=True)
    out = np.einsum("bhqk,bhkd->bhqd", attn, v)
    out = out.transpose(0, 2, 1, 3).reshape(B, S, D)
    return (out @ w_o.T + b_o).astype(np.float32)


def kernel(x, attention_mask, w_q, b_q, w_k, b_k, w_v, b_v, w_o, b_o,
           _debug=False, _trace=False):
    x = np.asarray(x, np.float32)
    attention_mask = np.asarray(attention_mask)
    if not np.all(attention_mask != 0):
        return _numpy_reference(np.asarray(x), np.asarray(attention_mask),
                                *[np.asarray(a) for a in
                                  (w_q, b_q, w_k, b_k, w_v, b_v, w_o, b_o)])
    w_q, w_k, w_v, w_o = [np.asarray(w, np.float32) for w in (w_q, w_k, w_v, w_o)]
    b_q, b_k, b_v, b_o = [np.asarray(b, np.float32) for b in (b_q, b_k, b_v, b_o)]

    nc = _get_nc(_debug)
    in_maps = _prep_inputs(x, w_q, b_q, w_k, w_v)
    woT_f = np.ascontiguousarray(w_o.T).astype(NPBF)
    for core in range(N_CORES):
        hg = core % 4
        in_maps[core]["woT"] = np.ascontiguousarray(
            woT_f[hg * 256:(hg + 1) * 256, :])

    res = run_bass_kernel_spmd(nc, in_maps, list(range(N_CORES)), trace=_trace)
    const_row = (b_v @ w_o.T + b_o).astype(np.float32)
    y = np.zeros((B, S, D), np.float32)
    for core in range(N_CORES):
        b = core // 4
        y[b] += res.results[core]["y"].astype(np.float32)
    y += const_row
    if _debug or _trace:
        return y, res
    return y


# revision 16
# speedup vs baseline: 1.0199x; 1.0199x over previous
"""Multi-head causal self-attention (B=2, S=2048, D=1024, H=16) on 8 TRN2 cores.

Sharding: core = b*4 + hg  (b in {0,1} batch, hg in {0..3} head-group of 4 heads).
Per core: project qT/kT (pair-packed [128, S], bf16) and v ([S, 64] blocks, bf16),
compute transposed scores S^T = K Q^T per head (k on partitions), causal mask
added in PSUM via a -1e9-triangle accumulate-matmul (PE, replaces the DVE
tri-multiply), exp on ScalarE (bf16 out, one call per k-tile covering both
packed heads via a 3D AP), PV matmul with a ones-column appended to V so row 64
of the accumulator is the softmax sum, normalization = fp32 reciprocal straight
from PSUM + bf16 numerator cast + partition-broadcast multiply, then the
partial output projection streamed out per 128-token block. All tile pools are
opened once (no mid-kernel pool barriers); PSUM: 2x scores (2 banks) + 2 o_ps
(1 bank) + 2 shared projection banks. Host sums the 4 per-batch partials and
adds (b_v @ w_o.T + b_o); b_k is dropped (softmax invariant to per-query
constants); b_q applied on-device. Matmul operands bf16; accumulation fp32.
"""

import numpy as np
import ml_dtypes

import concourse.bass as bass
import concourse.mybir as mybir
import concourse.tile as tile
from concourse import bacc
from concourse.bass_utils import run_bass_kernel_spmd

B, S, D, H, DK = 2, 2048, 1024, 16, 64
N_CORES = 8
F32 = mybir.dt.float32
BF16 = mybir.dt.bfloat16
NPBF = ml_dtypes.bfloat16
AF = mybir.ActivationFunctionType
NEG_BIG = -1.0e9
V2_NORM = True     # norm path with bf16 numerator + mixed-dtype multiply
V2_MASK = True     # causal mask via PE accumulate-matmul
V2_EXP3D = True    # single 3D-AP exp call on diag tiles


def _build(debug=False):
    nc = bacc.Bacc("TRN2", target_bir_lowering=False, debug=False,
                   num_devices=N_CORES)
    xT = nc.dram_tensor("xT", [D, S], BF16, kind="ExternalInput").ap()
    wqT = nc.dram_tensor("wqT", [D, 256], BF16, kind="ExternalInput").ap()
    wkT = nc.dram_tensor("wkT", [D, 256], BF16, kind="ExternalInput").ap()
    wvT = nc.dram_tensor("wvT", [D, 256], BF16, kind="ExternalInput").ap()
    woT = nc.dram_tensor("woT", [256, D], BF16, kind="ExternalInput").ap()
    bq2 = nc.dram_tensor("bq2", [128, 2], F32, kind="ExternalInput").ap()
    mtri = nc.dram_tensor("mtri", [128, 128], BF16, kind="ExternalInput").ap()
    ident = nc.dram_tensor("ident", [128, 128], BF16, kind="ExternalInput").ap()
    y = nc.dram_tensor("y", [S, D], BF16, kind="ExternalOutput").ap()
    dbg = {}
    if debug:
        for nm, shp in [("qT", [128, 2, S]), ("kT", [128, 2, S]),
                        ("vv", [128, 16, 260]), ("oT", [128, 2, S])]:
            dbg[nm] = nc.dram_tensor(nm, shp, BF16, kind="ExternalOutput").ap()

    NQC = 4          # q-chunks of 512
    QC = 512
    NKT = S // 128   # k tiles

    with tile.TileContext(nc) as tc, \
            nc.allow_low_precision(reason="bf16 attention kernel"):
        with (
            tc.tile_pool(name="persist", bufs=1) as persist,
            tc.tile_pool(name="kqv", bufs=2) as kqv,
            tc.tile_pool(name="xw", bufs=1) as xw,
            tc.tile_pool(name="ep", bufs=4) as ep,
            tc.tile_pool(name="rp", bufs=6) as rp,
            tc.tile_pool(name="fsb", bufs=4) as fsb,
            tc.tile_pool(name="sq", bufs=2, space="PSUM") as sqp,
            tc.tile_pool(name="ops", bufs=1, space="PSUM") as opp,
            tc.tile_pool(name="xps", bufs=2, space="PSUM") as xps,
        ):
            qT_sb = [kqv.tile([128, S], BF16, tag="qT", name=f"qT{p}") for p in range(2)]
            kT_sb = [kqv.tile([128, S], BF16, tag="kT", name=f"kT{p}") for p in range(2)]
            v_sb = [persist.tile([128, 4 * 65], BF16, tag=f"v{t}", name=f"v{t}")
                    for t in range(NKT)]
            outT_sb = [persist.tile([128, S], BF16, tag=f"oT{p}", name=f"oTs{p}")
                       for p in range(2)]
            wo_sb = [persist.tile([128, D], BF16, tag=f"wo{p}", name=f"wo{p}")
                     for p in range(2)]
            mask_sb = persist.tile([128, 128], BF16, tag="mtri")
            ident_sb = persist.tile([128, 128], BF16, tag="ident")
            bq_sb = persist.tile([128, 2], F32, tag="bq")

            xt = [xw.tile([128, S], BF16, tag=f"x{c}", name=f"xt{c}") for c in range(8)]
            wq_sb = [xw.tile([128, 256], BF16, tag=f"wq{c}", name=f"wqs{c}") for c in range(8)]
            wk_sb = [xw.tile([128, 256], BF16, tag=f"wk{c}", name=f"wks{c}") for c in range(8)]
            wv_sb = [xw.tile([128, 256], BF16, tag=f"wv{c}", name=f"wvs{c}") for c in range(8)]
            for c in range(8):
                nc.sync.dma_start(out=xt[c], in_=xT[c * 128:(c + 1) * 128, :])
            # pair-0 weight halves first; pair-1 halves + wo deprioritized
            for c in range(8):
                nc.scalar.dma_start(out=wq_sb[c][:, 0:128],
                                    in_=wqT[c * 128:(c + 1) * 128, 0:128])
                nc.scalar.dma_start(out=wk_sb[c][:, 0:128],
                                    in_=wkT[c * 128:(c + 1) * 128, 0:128])
            nc.gpsimd.dma_start(out=bq_sb, in_=bq2)
            nc.gpsimd.dma_start(out=mask_sb, in_=mtri)
            nc.gpsimd.dma_start(out=ident_sb, in_=ident)
            for c in range(8):
                nc.gpsimd.dma_start(out=wv_sb[c], in_=wvT[c * 128:(c + 1) * 128, :])
            for c in range(8):
                nc.scalar.dma_start(out=wq_sb[c][:, 128:256],
                                    in_=wqT[c * 128:(c + 1) * 128, 128:256])
                nc.scalar.dma_start(out=wk_sb[c][:, 128:256],
                                    in_=wkT[c * 128:(c + 1) * 128, 128:256])
            for p in range(2):
                nc.gpsimd.dma_start(out=wo_sb[p], in_=woT[p * 128:(p + 1) * 128, :])

            def qk_chain(p, j, which):
                ps = xps.tile([128, QC], F32, tag="pj", name="ps")
                w_sb = wq_sb if which == "q" else wk_sb
                for c in range(8):
                    nc.tensor.matmul(
                        ps, w_sb[c][:, p * 128:(p + 1) * 128],
                        xt[c][:, j * QC:(j + 1) * QC],
                        start=(c == 0), stop=(c == 7))
                if which == "q":
                    nc.vector.tensor_scalar_add(
                        qT_sb[p][:, j * QC:(j + 1) * QC], ps, bq_sb[:, p:p + 1])
                else:
                    nc.vector.tensor_copy(kT_sb[p][:, j * QC:(j + 1) * QC], ps)

            def v_chain(t):
                ps_v = xps.tile([128, QC], F32, tag="pj", name="ps_v")
                for c in range(8):
                    nc.tensor.matmul(
                        ps_v[:, 0:256], xt[c][:, t * 128:(t + 1) * 128], wv_sb[c],
                        start=(c == 0), stop=(c == 7))
                v_view = v_sb[t].rearrange("p (h w) -> p h w", w=65)
                nc.vector.memset(v_view[:, :, 64:65], 1.0)
                nc.vector.tensor_copy(
                    v_view[:, :, 0:64],
                    ps_v[:, 0:256].rearrange("p (h w) -> p h w", w=64))

            norm_rest = []

            def emit_norm(p_, q0_, o_ps_):
                for s in range(2):
                    t_cp = rp.tile([64, QC], BF16, tag="ocp", name="t_cp",
                                   bufs=4)
                    sums = rp.tile([1, QC], F32, tag="rc", name="sums",
                                   bufs=4)
                    if s == 1:
                        # offload to ScalarE so the two per-head norms
                        # evacuate PSUM in parallel
                        nc.scalar.activation(sums, o_ps_[s][64:65, :], AF.Copy)
                        nc.scalar.activation(t_cp, o_ps_[s][0:64, :], AF.Copy)
                    else:
                        nc.vector.tensor_copy(sums, o_ps_[s][64:65, :])
                        nc.vector.tensor_copy(t_cp, o_ps_[s][0:64, :])
                    norm_rest.append((p_, q0_, s, t_cp, sums))

            def make_bc(sums):
                recip = rp.tile([1, QC], F32, tag="recip2", name="recip")
                nc.vector.reciprocal_approx_fast(out=recip, in_=sums)
                bc = rp.tile([64, QC], F32, tag="bc", name="bc")
                nc.gpsimd.partition_broadcast(bc, recip)
                return bc

            def finish_norm(p_, q0_, s, t_cp, sums):
                bc = make_bc(sums)
                nc.vector.tensor_mul(
                    outT_sb[p_][s * 64:(s + 1) * 64, q0_:q0_ + QC],
                    t_cp, bc)

            def emit_pair(p, fillers):
                for qc in range(NQC):
                    q0 = qc * QC
                    nkt = 4 * qc + 4
                    o_ps = [opp.tile([65, QC], F32, tag=f"o{s}", name=f"ops{s}")
                            for s in range(2)]
                    pend = None
                    for kt in range(nkt):
                        o = kt * 128 - q0
                        diag = o >= 0
                        lo = o if diag else 0
                        s_ab = sqp.tile([128, 2 * QC], F32, tag="sq", name="s_ab")
                        for s in range(2):
                            half = s * QC
                            nc.tensor.matmul(
                                s_ab[:, half + lo:half + QC],
                                kT_sb[p][s * 64:(s + 1) * 64,
                                         kt * 128:(kt + 1) * 128],
                                qT_sb[p][s * 64:(s + 1) * 64,
                                         q0 + lo:q0 + QC],
                                start=True, stop=(not diag),
                                tile_position=(s * 64, 0),
                                skip_group_check=True)
                        if diag:
                            for s in range(2):
                                nc.tensor.matmul(
                                    s_ab[:, s * QC + o:s * QC + o + 128],
                                    mask_sb, ident_sb,
                                    start=False, stop=True,
                                    skip_group_check=True)
                        e_ab = ep.tile([128, 2 * QC], BF16, tag="e", name="e_ab")
                        if diag:
                            e3 = e_ab.rearrange("p (s n) -> p s n", n=QC)
                            s3 = s_ab.rearrange("p (s n) -> p s n", n=QC)
                            nc.scalar.activation(
                                e3[:, :, lo:QC], s3[:, :, lo:QC],
                                AF.Exp, scale=0.125)
                        else:
                            nc.scalar.activation(e_ab, s_ab, AF.Exp, scale=0.125)
                        if kt == 0:
                            while norm_rest:
                                finish_norm(*norm_rest.pop(0))
                        if fillers is not None:
                            fillers(qc, kt, nkt)
                        if pend is not None:
                            _kt, _e, _lo = pend
                            for s in range(2):
                                hb = 2 * p + s
                                nc.tensor.matmul(
                                    o_ps[s][:, _lo:QC],
                                    v_sb[_kt][:, hb * 65:(hb + 1) * 65],
                                    _e[:, s * QC + _lo:(s + 1) * QC],
                                    start=(_kt == 0), stop=False,
                                    skip_group_check=True)
                        pend = (kt, e_ab, lo)
                    _kt, _e, _lo = pend
                    for s in range(2):
                        hb = 2 * p + s
                        nc.tensor.matmul(
                            o_ps[s][:, _lo:QC],
                            v_sb[_kt][:, hb * 65:(hb + 1) * 65],
                            _e[:, s * QC + _lo:(s + 1) * QC],
                            start=False, stop=True,
                            skip_group_check=True)
                    emit_norm(p, q0, o_ps)
                    yield qc
                while norm_rest:
                    finish_norm(*norm_rest.pop(0))

            # ---- head: 6 chains (q/k j0+j1, v0-1) interleaved per x-chunk so
            # the PE tracks the incoming x DMA stream and warms the clock.
            # One accumulation chain per PSUM bank (start=True clears the
            # whole bank's has_written bits). ----
            hq = sqp.tile([128, 2 * QC], F32, tag="sq", name="hq")  # q0 | k0
            hv = sqp.tile([128, 2 * QC], F32, tag="sq", name="hv")  # v0 | v1
            hx = xps.tile([128, QC], F32, tag="pj", name="hx")      # q j1
            hy = xps.tile([128, QC], F32, tag="pj", name="hy")      # k j1
            for c in range(8):
                st, sp = (c == 0), (c == 7)
                nc.tensor.matmul(hq[:, 0:QC], wq_sb[c][:, 0:128],
                                 xt[c][:, 0:QC], start=st, stop=sp)
                nc.tensor.matmul(hq[:, QC:2 * QC], wk_sb[c][:, 0:128],
                                 xt[c][:, 0:QC], start=st, stop=sp)
                for t in range(2):
                    nc.tensor.matmul(hv[:, t * QC:t * QC + 256],
                                     xt[c][:, t * 128:(t + 1) * 128],
                                     wv_sb[c], start=st, stop=sp)
                nc.tensor.matmul(hx, wq_sb[c][:, 0:128],
                                 xt[c][:, QC:2 * QC], start=st, stop=sp)
                nc.tensor.matmul(hy, wk_sb[c][:, 0:128],
                                 xt[c][:, QC:2 * QC], start=st, stop=sp)
            nc.vector.tensor_scalar_add(qT_sb[0][:, 0:QC], hq[:, 0:QC],
                                        bq_sb[:, 0:1])
            nc.vector.tensor_copy(kT_sb[0][:, 0:QC], hq[:, QC:2 * QC])
            for t in range(2):
                v_view = v_sb[t].rearrange("p (h w) -> p h w", w=65)
                nc.vector.memset(v_view[:, :, 64:65], 1.0)
                nc.vector.tensor_copy(
                    v_view[:, :, 0:64],
                    hv[:, t * QC:t * QC + 256].rearrange(
                        "p (h w) -> p h w", w=64))
            nc.vector.tensor_scalar_add(qT_sb[0][:, QC:2 * QC], hx,
                                        bq_sb[:, 0:1])
            nc.vector.tensor_copy(kT_sb[0][:, QC:2 * QC], hy)

            # ---- pair 0; remaining projections as fillers ----
            # qc0: v2-7, qc1: j2+v8-11, qc2: j3+v12-15,
            # qc3: pair-1 j0+j1 projection chains
            sched = {0: [], 1: [], 2: [], 3: []}
            for t in range(2, 8):
                sched[0].append(lambda t=t: v_chain(t))
            for j in (2, 3):
                sched[j - 1].append(lambda j=j: qk_chain(0, j, "q"))
                sched[j - 1].append(lambda j=j: qk_chain(0, j, "k"))
                for t in range(4 * j, 4 * j + 4):
                    sched[j - 1].append(lambda t=t: v_chain(t))
            for j in range(2):
                sched[3].append(lambda j=j: qk_chain(1, j, "q"))
                sched[3].append(lambda j=j: qk_chain(1, j, "k"))
            queues = [list(sched[qc]) for qc in range(4)]

            def filler_pop(qc, kt, nkt):
                q = queues[qc]
                rem_slots = nkt - kt
                while q and len(q) >= rem_slots:
                    q.pop(0)()
                if q:
                    q.pop(0)()

            for _qc in emit_pair(0, filler_pop):
                while queues[_qc]:
                    queues[_qc].pop(0)()

            # ---- pair 1: pair-1 j2/j3 chains + output projection as fillers ----
            def c_unit(qt, oc):
                f_ps = xps.tile([128, QC], F32, tag="pj", name="f_ps")
                for p in range(2):
                    nc.tensor.matmul(
                        f_ps, outT_sb[p][:, qt * 128:(qt + 1) * 128],
                        wo_sb[p][:, oc * 512:(oc + 1) * 512],
                        start=(p == 0), stop=(p == 1))
                f_sb = fsb.tile([128, QC], BF16, tag="f", name="f_sb")
                nc.vector.tensor_copy(f_sb, f_ps)
                nc.sync.dma_start(
                    out=y[qt * 128:(qt + 1) * 128,
                          oc * 512:(oc + 1) * 512],
                    in_=f_sb)

            cqueues = [[] for _ in range(4)]
            for j in (2, 3):
                cqueues[j - 2].append(lambda j=j: qk_chain(1, j, "q"))
                cqueues[j - 2].append(lambda j=j: qk_chain(1, j, "k"))

            def c_pop(qc, kt, nkt):
                q = cqueues[qc]
                rem_slots = nkt - kt
                while q and len(q) >= rem_slots:
                    q.pop(0)()
                if q:
                    q.pop(0)()

            for qc in emit_pair(1, c_pop):
                units = []
                for qt in range(qc * 4, (qc + 1) * 4):
                    for oc in range(2):
                        units.append(
                            lambda qt=qt, oc=oc: c_unit(qt, oc))
                if qc < 3:
                    cqueues[qc + 1].extend(units)
                else:
                    # final chunk: reciprocal+broadcast once per head, then
                    # normalize per 128-token block and fire its output
                    # projection immediately so PE/DVE/DMA pipeline the tail
                    pre = []
                    while norm_rest:
                        p_, q0_, s, t_cp, sums = norm_rest.pop(0)
                        pre.append((p_, q0_, s, t_cp, make_bc(sums)))
                    for qt_i in range(4):
                        c0 = qt_i * 128
                        for (p_, q0_, s, t_cp, bc) in pre:
                            nc.vector.tensor_mul(
                                outT_sb[p_][s * 64:(s + 1) * 64,
                                            q0_ + c0:q0_ + c0 + 128],
                                t_cp[:, c0:c0 + 128], bc[:, c0:c0 + 128])
                        qt = 12 + qt_i
                        c_unit(qt, 0)
                        c_unit(qt, 1)
            for q in cqueues:
                while q:
                    q.pop(0)()

            if debug:
                for p in range(2):
                    nc.sync.dma_start(out=dbg["oT"][:, p, :], in_=outT_sb[p])
                    nc.sync.dma_start(out=dbg["qT"][:, p, :], in_=qT_sb[p])
                    nc.sync.dma_start(out=dbg["kT"][:, p, :], in_=kT_sb[p])
                for t in range(NKT):
                    nc.sync.dma_start(out=dbg["vv"][:, t, :], in_=v_sb[t])

    nc.compile()
    return nc


_cached = {}


def _get_nc(debug=False):
    key = bool(debug)
    if key not in _cached:
        _cached[key] = _build(debug)
    return _cached[key]


def _prep_inputs(x, w_q, b_q, w_k, w_v):
    mtri = (np.triu(np.ones((128, 128), np.float32), 1) * NEG_BIG).astype(NPBF)
    ident = np.eye(128, dtype=np.float32).astype(NPBF)
    wqT_f = np.ascontiguousarray(w_q.T).astype(NPBF)
    wkT_f = np.ascontiguousarray(w_k.T).astype(NPBF)
    wvT_f = np.ascontiguousarray(w_v.T).astype(NPBF)
    in_maps = []
    for core in range(N_CORES):
        b, hg = divmod(core, 4)
        cs = slice(hg * 256, (hg + 1) * 256)
        in_maps.append({
            "xT": np.ascontiguousarray(x[b].T).astype(NPBF),
            "wqT": np.ascontiguousarray(wqT_f[:, cs]),
            "wkT": np.ascontiguousarray(wkT_f[:, cs]),
            "wvT": np.ascontiguousarray(wvT_f[:, cs]),
            "bq2": np.ascontiguousarray(
                b_q[hg * 256:(hg + 1) * 256].reshape(2, 128).T.astype(np.float32)),
            "mtri": mtri,
            "ident": ident,
        })
    return in_maps


def _numpy_reference(x, attention_mask, w_q, b_q, w_k, b_k, w_v, b_v, w_o, b_o):
    x = x.astype(np.float64)
    q = (x @ w_q.T + b_q).reshape(B, S, H, DK).transpose(0, 2, 1, 3)
    k = (x @ w_k.T + b_k).reshape(B, S, H, DK).transpose(0, 2, 1, 3)
    v = (x @ w_v.T + b_v).reshape(B, S, H, DK).transpose(0, 2, 1, 3)
    scores = np.einsum("bhqd,bhkd->bhqk", q, k) / np.sqrt(DK)
    causal = np.tril(np.ones((S, S), bool))
    mask = causal[None, None] & (attention_mask[:, None, None, :] != 0)
    scores = np.where(mask, scores, -np.inf)
    scores -= scores.max(-1, keepdims=True)
    e = np.exp(scores)
    attn = e / e.sum(-1, keepdims=True)
    out = np.einsum("bhqk,bhkd->bhqd", attn, v)
    out = out.transpose(0, 2, 1, 3).reshape(B, S, D)
    return (out @ w_o.T + b_o).astype(np.float32)


def kernel(x, attention_mask, w_q, b_q, w_k, b_k, w_v, b_v, w_o, b_o,
           _debug=False, _trace=False):
    x = np.asarray(x, np.float32)
    attention_mask = np.asarray(attention_mask)
    if not np.all(attention_mask != 0):
        return _numpy_reference(np.asarray(x), np.asarray(attention_mask),
                                *[np.asarray(a) for a in
                                  (w_q, b_q, w_k, b_k, w_v, b_v, w_o, b_o)])
    w_q, w_k, w_v, w_o = [np.asarray(w, np.float32) for w in (w_q, w_k, w_v, w_o)]
    b_q, b_k, b_v, b_o = [np.asarray(b, np.float32) for b in (b_q, b_k, b_v, b_o)]

    nc = _get_nc(_debug)
    in_maps = _prep_inputs(x, w_q, b_q, w_k, w_v)
    woT_f = np.ascontiguousarray(w_o.T).astype(NPBF)
    for core in range(N_CORES):
        hg = core % 4
        in_maps[core]["woT"] = np.ascontiguousarray(
            woT_f[hg * 256:(hg + 1) * 256, :])

    res = run_bass_kernel_spmd(nc, in_maps, list(range(N_CORES)), trace=_trace)
    const_row = (b_v @ w_o.T + b_o).astype(np.float32)
    y = np.zeros((B, S, D), np.float32)
    for core in range(N_CORES):
        b = core // 4
        y[b] += res.results[core]["y"].astype(np.float32)
    y += const_row
    if _debug or _trace:
        return y, res
    return y
